# revision 1
# baseline (speedup 1.0000x reference)
"""Chamfer distance kernel for Trainium2 (8 NeuronCores).

Problem: pred/target [4, 8192, 3] f32 -> scalar
  mean_b( mean_m min_n ||p_bm - q_bn||^2 + mean_n min_m ||p_bm - q_bn||^2 )

Strategy (one "side" per core; 4 batches x 2 directions = 8 cores):
  Each core owns one (batch, direction) pair and computes, for each of its
  8192 "own" points, the min squared distance to all 8192 "other" points.

  Distances are produced on the TensorEngine as K=8 matmuls using the
  identity ||p-q||^2 = -2 p.q + ||p||^2 + ||q||^2:
      lhsT rows: [-2x, -2y, -2z, n_hi, n_lo, 1, 1, 0]   (own points)
      rhs  rows: [ x,   y,  z,  1,    1,  n_hi, n_lo, 0] (other points)
  Inputs are fp16; norms are split hi/lo into two fp16 values so the norm
  contribution keeps ~2^-22 precision; fp16 products are exact in the fp32
  PSUM accumulation. Because K=8 uses only 8 of the PE's 128 rows — and
  this part runs the PE cold at 1.2 GHz — four matmuls are packed into
  disjoint 32-row groups via tile_position, running concurrently (~4x).
  Host-side prep replicates lhsT/rhs at partition offsets 0/32/64/96.

  Each m-tile (128 own points) streams 4 "rounds" of 2048 distances into
  two rotating [128,2048] PSUM tiles (4 banks each). ScalarE stages 3
  rounds to fp16 SBUF; VectorE pair-mins (PSUM,staged) at 1x and
  (staged,staged) at fp16 2x. The merge/fold ladder is batched across
  m-tile pairs, and the final fold+reduce across GRP=8 m-tiles, to
  amortize per-op overheads. The 3-staged/1-direct split balances
  ScalarE vs VectorE (both ~equally busy, measured).
"""

import numpy as np

import concourse.bacc as bacc
import concourse.mybir as mybir
import concourse.tile as tile
from concourse import bass_utils

P = 128          # partitions / m-tile size
NPTS = 8192      # points per cloud
B = 4            # batch
K = 8            # matmul contraction (padded)
MT = NPTS // P   # 64 m-tiles
RND = 2048       # columns per round (one 4-bank PSUM tile, 4 packed MMs)
MM_N = 512       # matmul free dim (one PSUM bank of fp32)
GRP = 8          # m-tiles whose tails are batched into one fold+reduce

F16 = mybir.dt.float16
F32 = mybir.dt.float32
MIN = mybir.AluOpType.min


def _emit_round(nc, ps, lt4, rt4, t, r):
    """4 row-group-packed K=8 matmuls filling one [128, 2048] PSUM tile."""
    for i in range(4):
        n0 = r * RND + i * MM_N
        nc.tensor.matmul(
            ps[:, i * MM_N:(i + 1) * MM_N],
            lt4[32 * i:32 * i + K, t * P:(t + 1) * P],
            rt4[32 * i:32 * i + K, n0:n0 + MM_N],
            start=True,
            stop=True,
            tile_position=(32 * i, 0),
        )


def _emit_mtile_l1(nc, psum, stg, lt4, rt4, t, x0_slot, x1_slot):
    """One m-tile's matmuls + L1 pair-mins into the pair buffers.

    Rounds 0-2 are staged to fp16 SBUF by ScalarE; round 3 is drained by
    VectorE against the early-staged st0 (1x); st1/st2 pair at fp16 2x.
    """
    staged = []
    for r in range(3):
        ps = psum.tile([P, RND], F32, tag="ps")
        _emit_round(nc, ps, lt4, rt4, t, r)
        st = stg.tile([P, RND], F16, tag="st")
        nc.scalar.copy(st[:], ps[:])
        staged.append(st)
    ps3 = psum.tile([P, RND], F32, tag="ps")
    _emit_round(nc, ps3, lt4, rt4, t, 3)

    nc.vector.tensor_tensor(x0_slot, ps3[:], staged[0][:], op=MIN)
    nc.vector.tensor_tensor(x1_slot, staged[1][:], staged[2][:], op=MIN)


def _build_nc():
    nc = bacc.Bacc(
        "TRN2", target_bir_lowering=False, debug=False, num_devices=8
    )
    lhsT_d = nc.dram_tensor("lhsT", [P, NPTS], F16, kind="ExternalInput")
    rhs_d = nc.dram_tensor("rhs", [P, NPTS], F16, kind="ExternalInput")
    mins_d = nc.dram_tensor("mins", [P, MT], F32, kind="ExternalOutput")

    with tile.TileContext(nc) as tc:
        with (
            tc.tile_pool(name="const", bufs=1) as const,
            tc.tile_pool(name="psum", bufs=2, space="PSUM") as psum,
            tc.tile_pool(name="stg", bufs=6) as stg,
            tc.tile_pool(name="xpool", bufs=3) as xpool,
            tc.tile_pool(name="wpool", bufs=2) as wpool,
        ):
            lt4 = const.tile([P, NPTS], F16)
            rt4 = const.tile([P, NPTS], F16)
            res = const.tile([P, MT], F32)
            # first m-tile's weights + first rounds' rhs land first so the
            # PE starts streaming before the bulk of the input DMA finishes
            nc.sync.dma_start(lt4[:, :P], lhsT_d.ap()[:, :P])
            nc.sync.dma_start(rt4[:, :RND], rhs_d.ap()[:, :RND])
            nc.sync.dma_start(lt4[:, P:], lhsT_d.ap()[:, P:])
            nc.sync.dma_start(rt4[:, RND:], rhs_d.ap()[:, RND:])

            W = MM_N  # wbuf slot width (512)
            for g in range(MT // GRP):
                wbuf = wpool.tile([P, GRP, W], F16, tag="w")
                for j in range(GRP // 2):
                    # process an m-tile pair; batch its merge ladder
                    x0b = xpool.tile([P, 2, RND], F16, tag="x0")
                    x1b = xpool.tile([P, 2, RND], F16, tag="x1")
                    for h in range(2):
                        t = g * GRP + 2 * j + h
                        _emit_mtile_l1(
                            nc, psum, stg, lt4, rt4, t,
                            x0b[:, h, :], x1b[:, h, :],
                        )
                    z2 = xpool.tile([P, 2, RND], F16, tag="z2")
                    nc.vector.tensor_tensor(z2[:], x0b[:], x1b[:], op=MIN)
                    zz2 = xpool.tile([P, 2, RND // 2], F16, tag="zz2")
                    nc.vector.tensor_tensor(
                        zz2[:], z2[:, :, :RND // 2], z2[:, :, RND // 2:],
                        op=MIN,
                    )
                    nc.vector.tensor_tensor(
                        wbuf[:, 2 * j:2 * j + 2, :],
                        zz2[:, :, :RND // 4], zz2[:, :, RND // 4:], op=MIN,
                    )
                # batched tail: fold 512->256->128, reduce 128->1 per m-tile
                v = wpool.tile([P, GRP, W // 2], F16, tag="v")
                nc.vector.tensor_tensor(
                    v[:], wbuf[:, :, :W // 2], wbuf[:, :, W // 2:], op=MIN
                )
                u = wpool.tile([P, GRP, W // 4], F16, tag="u")
                nc.vector.tensor_tensor(
                    u[:], v[:, :, :W // 4], v[:, :, W // 4:], op=MIN
                )
                nc.vector.tensor_reduce(
                    res[:, g * GRP:(g + 1) * GRP], u[:],
                    axis=mybir.AxisListType.X, op=MIN,
                )

            nc.sync.dma_start(mins_d.ap(), res[:])

    nc.compile()
    return nc


_NC_CACHE = []


def _get_nc():
    if not _NC_CACHE:
        _NC_CACHE.append(_build_nc())
    return _NC_CACHE[0]


def _prep_side(own, other):
    """Build lhsT [128, N] and rhs [128, N] fp16 with the K=8 row content
    replicated at partition offsets 0/32/64/96 for row-group packing."""
    o16 = own.astype(np.float16)
    t16 = other.astype(np.float16)
    o32 = o16.astype(np.float32)
    t32 = t16.astype(np.float32)
    on = (o32 * o32).sum(-1)       # fp32 norms of the fp16-rounded points
    tn = (t32 * t32).sum(-1)
    on_hi = on.astype(np.float16)
    on_lo = (on - on_hi.astype(np.float32)).astype(np.float16)
    tn_hi = tn.astype(np.float16)
    tn_lo = (tn - tn_hi.astype(np.float32)).astype(np.float16)

    n = own.shape[0]
    lhsT = np.zeros((K, n), np.float16)
    lhsT[0:3] = (-2.0 * o32).astype(np.float16).T
    lhsT[3] = on_hi
    lhsT[4] = on_lo
    lhsT[5] = 1.0
    lhsT[6] = 1.0
    rhs = np.zeros((K, n), np.float16)
    rhs[0:3] = t16.T
    rhs[3] = 1.0
    rhs[4] = 1.0
    rhs[5] = tn_hi
    rhs[6] = tn_lo

    lhsT4 = np.zeros((P, n), np.float16)
    rhs4 = np.zeros((P, n), np.float16)
    for g in range(4):
        lhsT4[32 * g:32 * g + K] = lhsT
        rhs4[32 * g:32 * g + K] = rhs
    return lhsT4, rhs4


def _in_maps_for(pred, target):
    pred = np.asarray(pred, dtype=np.float32)
    target = np.asarray(target, dtype=np.float32)
    in_maps = []
    for b in range(B):
        for d in range(2):
            own, other = (
                (pred[b], target[b]) if d == 0 else (target[b], pred[b])
            )
            lhsT4, rhs4 = _prep_side(own, other)
            in_maps.append({"lhsT": lhsT4, "rhs": rhs4})
    return in_maps


def kernel(pred, target):
    in_maps = _in_maps_for(pred, target)
    nc = _get_nc()
    r = bass_utils.run_bass_kernel_spmd(nc, in_maps, core_ids=list(range(8)))

    total = 0.0
    for core_res in r.results:
        total += core_res["mins"].astype(np.float64).mean()
    return np.array(total / B, dtype=np.float32)



# revision 6
# speedup vs baseline: 4.5159x; 4.5159x over previous
"""Chamfer distance kernel for Trainium2 (8 NeuronCores).

Problem: pred/target [4, 8192, 3] f32 -> scalar
  mean_b( mean_m min_n ||p_bm - q_bn||^2 + mean_n min_m ||p_bm - q_bn||^2 )

Strategy (one "side" per core; 4 batches x 2 directions = 8 cores):
  Each core owns one (batch, direction) pair. Both clouds are sorted on the
  host along one coordinate axis; each m-tile of 128 "own" points only
  computes distances to a static window of WP=1024 "other" points centered
  at the matching rank (rank-locality of nearest neighbors in a sorted
  cloud). The result is certified exactly on the host: a point whose
  windowed min distance is smaller than the squared distance from its sort
  key to the window's edge key provably has its true NN inside the window;
  the handful of uncertified points (~30/cloud) get an exact O(N) host
  recheck. This cuts device work 8x versus the full 8192x8192 sweep while
  keeping the result exact up to fp16 rounding of the certified mins.

  Distances are produced on the TensorEngine as K=8 matmuls using the
  identity ||p-q||^2 = -2 p.q + ||p||^2 + ||q||^2:
      lhsT rows: [-2x, -2y, -2z, n_hi, n_lo, 1, 1, 0]   (own points)
      rhs  rows: [ x,   y,  z,  1,    1,  n_hi, n_lo, 0] (other points)
  Inputs are fp16; norms are split hi/lo into two fp16 values so the norm
  contribution keeps ~2^-22 precision; fp16 products are exact in the fp32
  PSUM accumulation. Four matmuls are packed into disjoint 32-row groups
  via tile_position (host prep replicates lhsT/rhs at partition offsets
  0/32/64/96); matmul PSUM writes must be bank-aligned (512 fp32), so one
  [128, 2048] PSUM tile holds TWO m-tiles' windows (two 512-wide
  bank-aligned outs per m-tile).

  Drain (the DVE can read at most ONE PSUM operand per instruction, and
  tensor_tensor_reduce min/min is broken on this hardware): ScalarE copies
  the window's first half to fp16 SBUF while VectorE pair-mins the second
  half against it (one PSUM read + one SBUF read per lane-cycle). The
  surviving 512 fp16 mins per m-tile are folded 512->64 at fp16 2x and
  min-reduced, batched across GRP=8 m-tiles to amortize per-op overheads.
"""

import numpy as np

import concourse.bacc as bacc
import concourse.mybir as mybir
import concourse.tile as tile
from concourse import bass_utils

P = 128          # partitions / m-tile size
NPTS = 8192      # points per cloud
B = 4            # batch
K = 8            # matmul contraction (padded)
MT = NPTS // P   # 64 m-tiles
WP = 1024        # window columns per m-tile
HALF = WP // 2
MM_N = WP // 4   # matmul free dim per packed group (256)
GRP = 8          # m-tiles whose tails are batched into one fold+reduce

F16 = mybir.dt.float16
F32 = mybir.dt.float32
MIN = mybir.AluOpType.min


def _win_start(t):
    return min(max(t * P + P // 2 - WP // 2, 0), NPTS - WP)


def _build_nc():
    nc = bacc.Bacc(
        "TRN2", target_bir_lowering=False, debug=False, num_devices=8
    )
    lhsT_d = nc.dram_tensor("lhsT", [P, NPTS], F16, kind="ExternalInput")
    rhs_d = nc.dram_tensor("rhs", [P, NPTS], F16, kind="ExternalInput")
    mins_d = nc.dram_tensor("mins", [P, MT], F32, kind="ExternalOutput")

    with tile.TileContext(nc) as tc:
        with (
            tc.tile_pool(name="const", bufs=1) as const,
            tc.tile_pool(name="psum", bufs=2, space="PSUM") as psum,
            tc.tile_pool(name="stg", bufs=4) as stg,
            tc.tile_pool(name="xpool", bufs=2) as xpool,
            tc.tile_pool(name="wpool", bufs=2) as wpool,
        ):
            lt4 = const.tile([P, NPTS], F16)
            rt4 = const.tile([P, NPTS], F16)
            res = const.tile([P, MT], F32)
            # first m-tile's weights + window land first so the PE starts
            # streaming before the bulk of the input DMA finishes
            nc.sync.dma_start(lt4[:, :P], lhsT_d.ap()[:, :P])
            nc.sync.dma_start(rt4[:, :WP], rhs_d.ap()[:, :WP])
            nc.sync.dma_start(lt4[:, P:], lhsT_d.ap()[:, P:])
            nc.sync.dma_start(rt4[:, WP:], rhs_d.ap()[:, WP:])

            for g in range(MT // GRP):
                xb = xpool.tile([P, GRP, HALF], F16, tag="xb")
                for jp in range(GRP // 2):
                    ps = psum.tile([P, 2 * WP], F32, tag="ps")
                    for h in range(2):
                        t = g * GRP + 2 * jp + h
                        S = _win_start(t)
                        for i in range(2):
                            grp = 2 * h + i
                            nc.tensor.matmul(
                                ps[:, (2 * h + i) * HALF:
                                      (2 * h + i + 1) * HALF],
                                lt4[32 * grp:32 * grp + K,
                                    t * P:(t + 1) * P],
                                rt4[32 * grp:32 * grp + K,
                                    S + i * HALF:S + (i + 1) * HALF],
                                start=True,
                                stop=True,
                                tile_position=(32 * grp, 0),
                            )
                    for h in range(2):
                        j = 2 * jp + h
                        st = stg.tile([P, HALF], F16, tag="st")
                        nc.scalar.copy(st[:], ps[:, 2 * h * HALF:
                                                   (2 * h + 1) * HALF])
                        nc.vector.tensor_tensor(
                            xb[:, j, :],
                            ps[:, (2 * h + 1) * HALF:(2 * h + 2) * HALF],
                            st[:], op=MIN,
                        )
                # batched tail: fold 512->64 at fp16 2x, min-reduce 64->1
                v1 = wpool.tile([P, GRP, HALF // 2], F16, tag="v1")
                nc.vector.tensor_tensor(
                    v1[:], xb[:, :, :HALF // 2], xb[:, :, HALF // 2:], op=MIN
                )
                v2 = wpool.tile([P, GRP, HALF // 4], F16, tag="v2")
                nc.vector.tensor_tensor(
                    v2[:], v1[:, :, :HALF // 4], v1[:, :, HALF // 4:], op=MIN
                )
                v3 = wpool.tile([P, GRP, HALF // 8], F16, tag="v3")
                nc.vector.tensor_tensor(
                    v3[:], v2[:, :, :HALF // 8], v2[:, :, HALF // 8:], op=MIN
                )
                nc.vector.tensor_reduce(
                    res[:, g * GRP:(g + 1) * GRP], v3[:],
                    axis=mybir.AxisListType.X, op=MIN,
                )

            nc.sync.dma_start(mins_d.ap(), res[:])

    nc.compile()
    return nc


_NC_CACHE = []


def _get_nc():
    if not _NC_CACHE:
        _NC_CACHE.append(_build_nc())
    return _NC_CACHE[0]


def _prep_side(own, other):
    """Build lhsT [128, N] and rhs [128, N] fp16 with the K=8 row content
    replicated at partition offsets 0/32/64/96 for row-group packing."""
    o16 = own.astype(np.float16)
    t16 = other.astype(np.float16)
    o32 = o16.astype(np.float32)
    t32 = t16.astype(np.float32)
    on = (o32 * o32).sum(-1)       # fp32 norms of the fp16-rounded points
    tn = (t32 * t32).sum(-1)
    on_hi = on.astype(np.float16)
    on_lo = (on - on_hi.astype(np.float32)).astype(np.float16)
    tn_hi = tn.astype(np.float16)
    tn_lo = (tn - tn_hi.astype(np.float32)).astype(np.float16)

    n = own.shape[0]
    lhsT = np.zeros((K, n), np.float16)
    lhsT[0:3] = (-2.0 * o32).astype(np.float16).T
    lhsT[3] = on_hi
    lhsT[4] = on_lo
    lhsT[5] = 1.0
    lhsT[6] = 1.0
    rhs = np.zeros((K, n), np.float16)
    rhs[0:3] = t16.T
    rhs[3] = 1.0
    rhs[4] = 1.0
    rhs[5] = tn_hi
    rhs[6] = tn_lo

    lhsT4 = np.zeros((P, n), np.float16)
    rhs4 = np.zeros((P, n), np.float16)
    for g in range(4):
        lhsT4[32 * g:32 * g + K] = lhsT
        rhs4[32 * g:32 * g + K] = rhs
    return lhsT4, rhs4


def _sides(pred, target):
    """Per-core (own_sorted, other_sorted, axis) for the 8 (batch,
    direction) pairs, with both clouds sorted along the batch's
    max-variance axis."""
    pred = np.asarray(pred, dtype=np.float32)
    target = np.asarray(target, dtype=np.float32)
    sides = []
    for b in range(B):
        axis = int(np.argmax(pred[b].var(0) + target[b].var(0)))
        for d in range(2):
            own, other = (
                (pred[b], target[b]) if d == 0 else (target[b], pred[b])
            )
            so = np.argsort(own[:, axis], kind="stable")
            st = np.argsort(other[:, axis], kind="stable")
            sides.append((own[so], other[st], axis))
    return sides


def _in_maps_for(pred, target):
    in_maps = []
    for own_s, oth_s, _axis in _sides(pred, target):
        lhsT4, rhs4 = _prep_side(own_s, oth_s)
        in_maps.append({"lhsT": lhsT4, "rhs": rhs4})
    return in_maps


def kernel(pred, target):
    sides = _sides(pred, target)
    in_maps = []
    for own_s, oth_s, _axis in sides:
        lhsT4, rhs4 = _prep_side(own_s, oth_s)
        in_maps.append({"lhsT": lhsT4, "rhs": rhs4})
    nc = _get_nc()
    r = bass_utils.run_bass_kernel_spmd(nc, in_maps, core_ids=list(range(8)))

    total = 0.0
    for core_res, (own_s, oth_s, axis) in zip(r.results, sides):
        # res[p, t] is the windowed min for sorted-own point t*128+p
        w = core_res["mins"].astype(np.float64).T.reshape(-1)
        # certification: window covers the true NN unless the windowed min
        # exceeds the squared key-distance to the window edge
        okey = oth_s[:, axis].astype(np.float64)
        own_key = own_s[:, axis].astype(np.float64)
        g = np.empty(NPTS)
        for t in range(MT):
            S = _win_start(t)
            ok = own_key[t * P:(t + 1) * P]
            gl = np.inf if S == 0 else ok - okey[S]
            gr = np.inf if S + WP == NPTS else okey[S + WP - 1] - ok
            g[t * P:(t + 1) * P] = np.minimum(gl, gr)
        uncert = np.nonzero(w > 0.98 * g * g)[0]
        if uncert.size:
            d = own_s[uncert, None, :].astype(np.float64) - oth_s[None, :, :]
            w[uncert] = (d * d).sum(-1).min(1)
        total += w.mean()
    return np.array(total / B, dtype=np.float32)


# revision 7
# speedup vs baseline: 7.1803x; 1.5900x over previous
"""Chamfer distance kernel for Trainium2 (8 NeuronCores).

Problem: pred/target [4, 8192, 3] f32 -> scalar
  mean_b( mean_m min_n ||p_bm - q_bn||^2 + mean_n min_m ||p_bm - q_bn||^2 )

Strategy (one "side" per core; 4 batches x 2 directions = 8 cores):
  Each core owns one (batch, direction) pair. Both clouds are sorted on the
  host along one coordinate axis; each m-tile of 128 "own" points only
  computes distances to a static window of WP=512 "other" points centered
  at the matching rank (rank-locality of nearest neighbors in a sorted
  cloud). The result is certified exactly on the host: a point whose
  windowed min distance is smaller than the squared distance from its sort
  key to the window's edge key provably has its true NN inside the window;
  the ~70 uncertified points per cloud get an exact O(N) host recheck.
  This cuts device work 16x versus the full 8192x8192 sweep while keeping
  the result exact up to fp16 rounding of the certified mins.

  Distances are produced on the TensorEngine as K=8 matmuls using the
  identity ||p-q||^2 = -2 p.q + ||p||^2 + ||q||^2:
      lhsT rows: [-2x, -2y, -2z, n_hi, n_lo, 1, 1, 0]   (own points)
      rhs  rows: [ x,   y,  z,  1,    1,  n_hi, n_lo, 0] (other points)
  Inputs are fp16; norms are split hi/lo into two fp16 values so the norm
  contribution keeps ~2^-22 precision; fp16 products are exact in the fp32
  PSUM accumulation. The host sends only the K=8 rows; on-device DMAs
  replicate them to partition offsets 0/32/64/96 so four matmuls pack into
  disjoint 32-row groups via tile_position. One [128, 2048] PSUM tile
  holds FOUR m-tiles' windows (one bank-aligned 512-wide out each).

  Drain (the DVE can read at most ONE PSUM operand per instruction):
  3 of every 8 m-tiles are min-reduced straight out of PSUM by the DVE
  (tensor_reduce, fp32); the other 5 are copied to fp16 SBUF by ScalarE
  and folded 512->64 at fp16 2x + min-reduced by the DVE, batched per
  group of 8 m-tiles. This balances ScalarE and VectorE at ~0.45us per
  m-tile each.
"""

import numpy as np

import concourse.bacc as bacc
import concourse.mybir as mybir
import concourse.tile as tile
from concourse import bass_utils

P = 128          # partitions / m-tile size
NPTS = 8192      # points per cloud
B = 4            # batch
K = 8            # matmul contraction (padded)
MT = NPTS // P   # 64 m-tiles
WP = 512         # window columns per m-tile (one PSUM bank)
GRP = 8          # m-tiles per drain-balancing group (2 PSUM quads)
DIRECT = (1, 3, 5)            # j-in-group drained by DVE straight from PSUM
ASSIST = (0, 2, 4, 6, 7)      # j-in-group staged by ScalarE + ladder
ND = len(DIRECT)
NA = len(ASSIST)
DCHUNK = 1024    # input DMA chunk columns (pipelines input vs compute)

F16 = mybir.dt.float16
F32 = mybir.dt.float32
MIN = mybir.AluOpType.min
AXX = None  # set below


def _win_start(t):
    return min(max(t * P + P // 2 - WP // 2, 0), NPTS - WP)


def _build_nc():
    nc = bacc.Bacc(
        "TRN2", target_bir_lowering=False, debug=False, num_devices=8
    )
    lhsT_d = nc.dram_tensor("lhsT", [K, NPTS], F16, kind="ExternalInput")
    rhs_d = nc.dram_tensor("rhs", [K, NPTS], F16, kind="ExternalInput")
    mind_d = nc.dram_tensor("mind", [P, ND * (MT // GRP)], F32,
                            kind="ExternalOutput")
    mina_d = nc.dram_tensor("mina", [P, NA * (MT // GRP)], F32,
                            kind="ExternalOutput")

    with tile.TileContext(nc) as tc:
        with (
            tc.tile_pool(name="const", bufs=1) as const,
            tc.tile_pool(name="psum", bufs=2, space="PSUM") as psum,
            tc.tile_pool(name="xpool", bufs=2) as xpool,
            tc.tile_pool(name="wpool", bufs=2) as wpool,
        ):
            lt4 = const.tile([P, NPTS], F16)
            rt4 = const.tile([P, NPTS], F16)
            res_d = const.tile([P, ND * (MT // GRP)], F32)
            res_a = const.tile([P, NA * (MT // GRP)], F32)
            # replicate the K=8 input rows to partition offsets 0/32/64/96
            # on-device (4x less HBM traffic), chunked so the first quads'
            # matmuls start before the bulk lands
            for c0 in range(0, NPTS, DCHUNK):
                for g4 in range(4):
                    nc.sync.dma_start(
                        lt4[32 * g4:32 * g4 + K, c0:c0 + DCHUNK],
                        lhsT_d.ap()[:, c0:c0 + DCHUNK],
                    )
                    nc.sync.dma_start(
                        rt4[32 * g4:32 * g4 + K, c0:c0 + DCHUNK],
                        rhs_d.ap()[:, c0:c0 + DCHUNK],
                    )

            for g in range(MT // GRP):
                xb = xpool.tile([P, NA, WP], F16, tag="xb")
                for q in range(2):
                    ps = psum.tile([P, 4 * WP], F32, tag="ps")
                    for u in range(4):
                        t = g * GRP + 4 * q + u
                        S = _win_start(t)
                        nc.tensor.matmul(
                            ps[:, u * WP:(u + 1) * WP],
                            lt4[32 * u:32 * u + K, t * P:(t + 1) * P],
                            rt4[32 * u:32 * u + K, S:S + WP],
                            start=True,
                            stop=True,
                            tile_position=(32 * u, 0),
                        )
                    for u in range(4):
                        j = 4 * q + u
                        t = g * GRP + j
                        sl = ps[:, u * WP:(u + 1) * WP]
                        if j in DIRECT:
                            di = g * ND + DIRECT.index(j)
                            nc.vector.tensor_reduce(
                                res_d[:, di:di + 1], sl,
                                axis=mybir.AxisListType.X, op=MIN,
                            )
                        else:
                            nc.scalar.copy(xb[:, ASSIST.index(j), :], sl)
                # batched tail: fold 512->64 at fp16 2x, min-reduce 64->1
                v1 = wpool.tile([P, NA, WP // 2], F16, tag="v1")
                nc.vector.tensor_tensor(
                    v1[:], xb[:, :, :WP // 2], xb[:, :, WP // 2:], op=MIN
                )
                v2 = wpool.tile([P, NA, WP // 4], F16, tag="v2")
                nc.vector.tensor_tensor(
                    v2[:], v1[:, :, :WP // 4], v1[:, :, WP // 4:], op=MIN
                )
                v3 = wpool.tile([P, NA, WP // 8], F16, tag="v3")
                nc.vector.tensor_tensor(
                    v3[:], v2[:, :, :WP // 8], v2[:, :, WP // 8:], op=MIN
                )
                nc.vector.tensor_reduce(
                    res_a[:, g * NA:(g + 1) * NA], v3[:],
                    axis=mybir.AxisListType.X, op=MIN,
                )

            nc.sync.dma_start(mind_d.ap(), res_d[:])
            nc.sync.dma_start(mina_d.ap(), res_a[:])

    nc.compile()
    return nc


_NC_CACHE = []


def _get_nc():
    if not _NC_CACHE:
        _NC_CACHE.append(_build_nc())
    return _NC_CACHE[0]


def _prep_side(own, other):
    """Build lhsT [8, N] and rhs [8, N] fp16 (K=8 rows; the device
    replicates them across partition offsets for row-group packing)."""
    o16 = own.astype(np.float16)
    t16 = other.astype(np.float16)
    o32 = o16.astype(np.float32)
    t32 = t16.astype(np.float32)
    on = (o32 * o32).sum(-1)       # fp32 norms of the fp16-rounded points
    tn = (t32 * t32).sum(-1)
    on_hi = on.astype(np.float16)
    on_lo = (on - on_hi.astype(np.float32)).astype(np.float16)
    tn_hi = tn.astype(np.float16)
    tn_lo = (tn - tn_hi.astype(np.float32)).astype(np.float16)

    n = own.shape[0]
    lhsT = np.zeros((K, n), np.float16)
    lhsT[0:3] = (-2.0 * o32).astype(np.float16).T
    lhsT[3] = on_hi
    lhsT[4] = on_lo
    lhsT[5] = 1.0
    lhsT[6] = 1.0
    rhs = np.zeros((K, n), np.float16)
    rhs[0:3] = t16.T
    rhs[3] = 1.0
    rhs[4] = 1.0
    rhs[5] = tn_hi
    rhs[6] = tn_lo
    return lhsT, rhs


def _sides(pred, target):
    """Per-core (own_sorted, other_sorted, axis) for the 8 (batch,
    direction) pairs, with both clouds sorted along the batch's
    max-variance axis."""
    pred = np.asarray(pred, dtype=np.float32)
    target = np.asarray(target, dtype=np.float32)
    sides = []
    for b in range(B):
        axis = int(np.argmax(pred[b].var(0) + target[b].var(0)))
        for d in range(2):
            own, other = (
                (pred[b], target[b]) if d == 0 else (target[b], pred[b])
            )
            so = np.argsort(own[:, axis], kind="stable")
            st = np.argsort(other[:, axis], kind="stable")
            sides.append((own[so], other[st], axis))
    return sides


def _in_maps_for(pred, target):
    in_maps = []
    for own_s, oth_s, _axis in _sides(pred, target):
        lhsT, rhs = _prep_side(own_s, oth_s)
        in_maps.append({"lhsT": lhsT, "rhs": rhs})
    return in_maps


def _assemble_mins(core_res):
    """[NPTS] windowed min per sorted-own point from the two outputs."""
    res_d = core_res["mind"].astype(np.float64)
    res_a = core_res["mina"].astype(np.float64)
    w = np.empty(NPTS)
    for g in range(MT // GRP):
        for j in range(GRP):
            t = g * GRP + j
            if j in DIRECT:
                col = res_d[:, g * ND + DIRECT.index(j)]
            else:
                col = res_a[:, g * NA + ASSIST.index(j)]
            w[t * P:(t + 1) * P] = col
    return w


def kernel(pred, target):
    sides = _sides(pred, target)
    in_maps = []
    for own_s, oth_s, _axis in sides:
        lhsT, rhs = _prep_side(own_s, oth_s)
        in_maps.append({"lhsT": lhsT, "rhs": rhs})
    nc = _get_nc()
    r = bass_utils.run_bass_kernel_spmd(nc, in_maps, core_ids=list(range(8)))

    total = 0.0
    for core_res, (own_s, oth_s, axis) in zip(r.results, sides):
        w = _assemble_mins(core_res)
        # certification: window covers the true NN unless the windowed min
        # exceeds the squared key-distance to the window edge
        okey = oth_s[:, axis].astype(np.float64)
        own_key = own_s[:, axis].astype(np.float64)
        g = np.empty(NPTS)
        for t in range(MT):
            S = _win_start(t)
            ok = own_key[t * P:(t + 1) * P]
            gl = np.inf if S == 0 else ok - okey[S]
            gr = np.inf if S + WP == NPTS else okey[S + WP - 1] - ok
            g[t * P:(t + 1) * P] = np.minimum(gl, gr)
        uncert = np.nonzero(w > 0.98 * g * g)[0]
        if uncert.size:
            d = own_s[uncert, None, :].astype(np.float64) - oth_s[None, :, :]
            w[uncert] = (d * d).sum(-1).min(1)
        total += w.mean()
    return np.array(total / B, dtype=np.float32)


# revision 13
# speedup vs baseline: 8.1839x; 1.1398x over previous
"""Chamfer distance kernel for Trainium2 (8 NeuronCores).

Problem: pred/target [4, 8192, 3] f32 -> scalar
  mean_b( mean_m min_n ||p_bm - q_bn||^2 + mean_n min_m ||p_bm - q_bn||^2 )

Strategy (one "side" per core; 4 batches x 2 directions = 8 cores):
  Each core owns one (batch, direction) pair. Both clouds are sorted on the
  host along one coordinate axis; each m-tile of 128 "own" points only
  computes distances to a static window of WP=512 "other" points centered
  at the matching rank (rank-locality of nearest neighbors in a sorted
  cloud). The result is certified exactly on the host: a point whose
  windowed min distance is smaller than the squared distance from its sort
  key to the window's edge key provably has its true NN inside the window;
  the ~70 uncertified points per cloud get an exact O(N) host recheck.
  This cuts device work 16x versus the full 8192x8192 sweep while keeping
  the result exact up to fp16 rounding of the certified mins.

  Distances are produced on the TensorEngine as K=8 matmuls using the
  identity ||p-q||^2 = -2 p.q + ||p||^2 + ||q||^2:
      lhsT rows: [-2x, -2y, -2z, n_hi, n_lo, 1, 1, 0]   (own points)
      rhs  rows: [ x,   y,  z,  1,    1,  n_hi, n_lo, 0] (other points)
  Inputs are fp16; norms are split hi/lo into two fp16 values so the norm
  contribution keeps ~2^-22 precision; fp16 products are exact in the fp32
  PSUM accumulation. Host prep replicates lhsT/rhs to partition offsets
  0/32/64/96 so four matmuls pack into disjoint 32-row groups via
  tile_position (full-width [128, n] DMAs; narrow [8, n] transfers only
  engage 8 of 128 SBUF lanes and pace the whole pipeline). Input DMA is
  chunked in 1024-column pieces so the first quads start ~1us in. One
  [128, 2048] PSUM tile holds FOUR m-tiles' windows (one bank-aligned
  512-wide out each).

  Drain (the DVE can read at most ONE PSUM operand per instruction):
  3 of every 8 m-tiles are min-reduced straight out of PSUM by the DVE
  (tensor_reduce, fp32); the other 5 are copied to fp16 SBUF by ScalarE
  and folded 512->64 at fp16 2x + min-reduced by the DVE, batched per
  group of 8 m-tiles. This balances ScalarE and VectorE at ~0.45us per
  m-tile each.
"""

import numpy as np

import concourse.bacc as bacc
import concourse.mybir as mybir
import concourse.tile as tile
from concourse import bass_utils

P = 128          # partitions / m-tile size
NPTS = 8192      # points per cloud
B = 4            # batch
K = 8            # matmul contraction (padded)
MT = NPTS // P   # 64 m-tiles
WP = 512         # window columns per m-tile (one PSUM bank)
GRP = 8          # m-tiles per drain-balancing group (2 PSUM quads)
DIRECT = (1, 3, 5)            # j-in-group drained by DVE straight from PSUM
ASSIST = (0, 2, 4, 6, 7)      # j-in-group staged by ScalarE + ladder
ND = len(DIRECT)
NA = len(ASSIST)
DCHUNK = 1024    # input DMA chunk columns (pipelines input vs compute)

F16 = mybir.dt.float16
F32 = mybir.dt.float32
MIN = mybir.AluOpType.min
AXX = None  # set below


def _win_start(t):
    return min(max(t * P + P // 2 - WP // 2, 0), NPTS - WP)


def _build_nc():
    nc = bacc.Bacc(
        "TRN2", target_bir_lowering=False, debug=False, num_devices=8
    )
    lhsT_d = nc.dram_tensor("lhsT", [P, NPTS], F16, kind="ExternalInput")
    rhs_d = nc.dram_tensor("rhs", [P, NPTS], F16, kind="ExternalInput")
    mind_d = nc.dram_tensor("mind", [P, ND * (MT // GRP)], F32,
                            kind="ExternalOutput")
    mina_d = nc.dram_tensor("mina", [P, NA * (MT // GRP)], F32,
                            kind="ExternalOutput")

    with tile.TileContext(nc) as tc:
        with (
            tc.tile_pool(name="const", bufs=1) as const,
            tc.tile_pool(name="psum", bufs=2, space="PSUM") as psum,
            tc.tile_pool(name="xpool", bufs=3) as xpool,
            tc.tile_pool(name="wpool", bufs=2) as wpool,
        ):
            lt4 = const.tile([P, NPTS], F16)
            rt4 = const.tile([P, NPTS], F16)
            res_d = const.tile([P, ND * (MT // GRP)], F32)
            res_a = const.tile([P, NA * (MT // GRP)], F32)
            # full-width chunked input DMA: the first quads' matmuls start
            # after the first ~256KB chunks land
            for c0 in range(0, NPTS, DCHUNK):
                nc.sync.dma_start(
                    rt4[:, c0:c0 + DCHUNK], rhs_d.ap()[:, c0:c0 + DCHUNK]
                )
                nc.sync.dma_start(
                    lt4[:, c0:c0 + DCHUNK], lhsT_d.ap()[:, c0:c0 + DCHUNK]
                )

            for g in range(MT // GRP):
                xb = xpool.tile([P, NA, WP], F16, tag="xb")
                for q in range(2):
                    ps = psum.tile([P, 4 * WP], F32, tag="ps")
                    for u in range(4):
                        t = g * GRP + 4 * q + u
                        S = _win_start(t)
                        nc.tensor.matmul(
                            ps[:, u * WP:(u + 1) * WP],
                            lt4[32 * u:32 * u + K, t * P:(t + 1) * P],
                            rt4[32 * u:32 * u + K, S:S + WP],
                            start=True,
                            stop=True,
                            tile_position=(32 * u, 0),
                        )
                    for u in range(4):
                        j = 4 * q + u
                        t = g * GRP + j
                        sl = ps[:, u * WP:(u + 1) * WP]
                        if j in DIRECT:
                            di = g * ND + DIRECT.index(j)
                            nc.vector.tensor_reduce(
                                res_d[:, di:di + 1], sl,
                                axis=mybir.AxisListType.X, op=MIN,
                            )
                        else:
                            nc.scalar.copy(xb[:, ASSIST.index(j), :], sl)
                # batched tail: fold 512->64 at fp16 2x, min-reduce 64->1
                v1 = wpool.tile([P, NA, WP // 2], F16, tag="v1")
                nc.vector.tensor_tensor(
                    v1[:], xb[:, :, :WP // 2], xb[:, :, WP // 2:], op=MIN
                )
                v2 = wpool.tile([P, NA, WP // 4], F16, tag="v2")
                nc.vector.tensor_tensor(
                    v2[:], v1[:, :, :WP // 4], v1[:, :, WP // 4:], op=MIN
                )
                v3 = wpool.tile([P, NA, WP // 8], F16, tag="v3")
                nc.vector.tensor_tensor(
                    v3[:], v2[:, :, :WP // 8], v2[:, :, WP // 8:], op=MIN
                )
                nc.vector.tensor_reduce(
                    res_a[:, g * NA:(g + 1) * NA], v3[:],
                    axis=mybir.AxisListType.X, op=MIN,
                )

            nc.sync.dma_start(mind_d.ap(), res_d[:])
            nc.sync.dma_start(mina_d.ap(), res_a[:])

    nc.compile()
    return nc


_NC_CACHE = []


def _get_nc():
    if not _NC_CACHE:
        _NC_CACHE.append(_build_nc())
    return _NC_CACHE[0]


def _prep_side(own, other):
    """Build lhsT [128, N] and rhs [128, N] fp16 with the K=8 row content
    replicated at partition offsets 0/32/64/96 for row-group packing."""
    o16 = own.astype(np.float16)
    t16 = other.astype(np.float16)
    o32 = o16.astype(np.float32)
    t32 = t16.astype(np.float32)
    on = (o32 * o32).sum(-1)       # fp32 norms of the fp16-rounded points
    tn = (t32 * t32).sum(-1)
    on_hi = on.astype(np.float16)
    on_lo = (on - on_hi.astype(np.float32)).astype(np.float16)
    tn_hi = tn.astype(np.float16)
    tn_lo = (tn - tn_hi.astype(np.float32)).astype(np.float16)

    n = own.shape[0]
    lhsT = np.zeros((K, n), np.float16)
    lhsT[0:3] = (-2.0 * o32).astype(np.float16).T
    lhsT[3] = on_hi
    lhsT[4] = on_lo
    lhsT[5] = 1.0
    lhsT[6] = 1.0
    rhs = np.zeros((K, n), np.float16)
    rhs[0:3] = t16.T
    rhs[3] = 1.0
    rhs[4] = 1.0
    rhs[5] = tn_hi
    rhs[6] = tn_lo

    lhsT4 = np.zeros((P, n), np.float16)
    rhs4 = np.zeros((P, n), np.float16)
    for g in range(4):
        lhsT4[32 * g:32 * g + K] = lhsT
        rhs4[32 * g:32 * g + K] = rhs
    return lhsT4, rhs4


def _sides(pred, target):
    """Per-core (own_sorted, other_sorted, axis) for the 8 (batch,
    direction) pairs, with both clouds sorted along the batch's
    max-variance axis."""
    pred = np.asarray(pred, dtype=np.float32)
    target = np.asarray(target, dtype=np.float32)
    sides = []
    for b in range(B):
        axis = int(np.argmax(pred[b].var(0) + target[b].var(0)))
        for d in range(2):
            own, other = (
                (pred[b], target[b]) if d == 0 else (target[b], pred[b])
            )
            so = np.argsort(own[:, axis], kind="stable")
            st = np.argsort(other[:, axis], kind="stable")
            sides.append((own[so], other[st], axis))
    return sides


def _in_maps_for(pred, target):
    in_maps = []
    for own_s, oth_s, _axis in _sides(pred, target):
        lhsT, rhs = _prep_side(own_s, oth_s)
        in_maps.append({"lhsT": lhsT, "rhs": rhs})
    return in_maps


def _assemble_mins(core_res):
    """[NPTS] windowed min per sorted-own point from the two outputs."""
    res_d = core_res["mind"].astype(np.float64)
    res_a = core_res["mina"].astype(np.float64)
    w = np.empty(NPTS)
    for g in range(MT // GRP):
        for j in range(GRP):
            t = g * GRP + j
            if j in DIRECT:
                col = res_d[:, g * ND + DIRECT.index(j)]
            else:
                col = res_a[:, g * NA + ASSIST.index(j)]
            w[t * P:(t + 1) * P] = col
    return w


def kernel(pred, target):
    sides = _sides(pred, target)
    in_maps = []
    for own_s, oth_s, _axis in sides:
        lhsT, rhs = _prep_side(own_s, oth_s)
        in_maps.append({"lhsT": lhsT, "rhs": rhs})
    nc = _get_nc()
    r = bass_utils.run_bass_kernel_spmd(nc, in_maps, core_ids=list(range(8)))

    total = 0.0
    for core_res, (own_s, oth_s, axis) in zip(r.results, sides):
        w = _assemble_mins(core_res)
        # certification: window covers the true NN unless the windowed min
        # exceeds the squared key-distance to the window edge
        okey = oth_s[:, axis].astype(np.float64)
        own_key = own_s[:, axis].astype(np.float64)
        g = np.empty(NPTS)
        for t in range(MT):
            S = _win_start(t)
            ok = own_key[t * P:(t + 1) * P]
            gl = np.inf if S == 0 else ok - okey[S]
            gr = np.inf if S + WP == NPTS else okey[S + WP - 1] - ok
            g[t * P:(t + 1) * P] = np.minimum(gl, gr)
        uncert = np.nonzero(w > 0.98 * g * g)[0]
        if uncert.size:
            d = own_s[uncert, None, :].astype(np.float64) - oth_s[None, :, :]
            w[uncert] = (d * d).sum(-1).min(1)
        total += w.mean()
    return np.array(total / B, dtype=np.float32)
